# revision 1
# baseline (speedup 1.0000x reference)
"""TRN2 Bass kernel for nn_BAKTSide (4-layer dense transformer, kq_same).

Sharding: data-parallel over batch across 8 NeuronCores (4 batches/core).
Layout: activations transposed (xT[d, tok]); matmuls bf16 on PE with fp32
PSUM accumulation. Attention per (batch, head) with scores in [j, i] layout
(symmetric since q == k); softmax normalizer Z comes free via a ones-column
appended to v; partition broadcasts via K=1 outer-product matmuls. The
residual master xT lives in DRAM (fp32) and is streamed per batch.
"""
import numpy as np
import ml_dtypes

import concourse.bass as bass
import concourse.bacc as bacc_mod
import concourse.mybir as mybir
from concourse.tile import TileContext
from concourse.bass_utils import run_bass_kernel_spmd

F32 = mybir.dt.float32
BF = mybir.dt.bfloat16
AF = mybir.ActivationFunctionType
OP = mybir.AluOpType

B, S, D, H, L, DFF = 32, 512, 1024, 16, 4, 2048
DK = D // H            # 64
NCH = D // 128         # 8
NFF = DFF // 128       # 16
NCORES = 8
BL = B // NCORES       # 4 batches per core
TOK = BL * S           # 2048 tokens per core
S4 = float(DK) ** -0.25
EPS = 1e-5
NEG = -1e38


def build(nc, L_run=L, BL_run=BL, dbg=None):
    tok = BL_run * S
    # ---------------- DRAM I/O ----------------
    qT_d = nc.dram_tensor("qT", [NCH, 128, tok], F32, kind="ExternalInput")
    qaT_d = nc.dram_tensor("qaT", [NCH, 128, tok], BF, kind="ExternalInput")
    wkbf = nc.dram_tensor("wk_t", [L, NCH, NCH, 128, 128], BF, kind="ExternalInput")
    wobf = nc.dram_tensor("wo_t", [L, NCH, NCH, 128, 128], BF, kind="ExternalInput")
    w1bf = nc.dram_tensor("w1_t", [L, NFF, NCH, 128, 128], BF, kind="ExternalInput")
    w2bf = nc.dram_tensor("w2_t", [L, NCH, NFF, 128, 128], BF, kind="ExternalInput")
    wvbf = nc.dram_tensor("wv_t", [L, NCH, 128, D], BF, kind="ExternalInput")
    pcol_d = nc.dram_tensor("pcol_h", [L, 128, 40], F32, kind="ExternalInput")
    prow_d = nc.dram_tensor("prow_h", [L, 1, 4 * D], BF, kind="ExternalInput")
    bvb_d = nc.dram_tensor("bvb_h", [L, 128, D], BF, kind="ExternalInput")
    masks_d = nc.dram_tensor("masks", [4, 128, S], BF, kind="ExternalInput")
    ones_d = nc.dram_tensor("ones", [128, S], BF, kind="ExternalInput")
    out_d = nc.dram_tensor("out", [NCH, 128, tok], F32, kind="ExternalOutput")
    dbg_d = (nc.dram_tensor("dbg", [128, NCH * S], F32, kind="ExternalOutput")
             if dbg else None)

    from contextlib import ExitStack
    with TileContext(nc) as tc, ExitStack() as stk:
        persist = stk.enter_context(tc.tile_pool(name="persist", bufs=1))
        dpool = stk.enter_context(tc.tile_pool(name="dram", bufs=1, space="DRAM"))
        lpar = stk.enter_context(tc.tile_pool(name="lparam", bufs=1))
        lnps = stk.enter_context(tc.tile_pool(name="lnps", bufs=3, space="PSUM"))

        masks = persist.tile([128, 4 * S], BF, tag="masks")
        ones = persist.tile([128, S], BF, tag="ones")
        eps_t = persist.tile([1, 1], F32, tag="eps")
        nc.vector.memset(eps_t[:], EPS)

        nc.sync.dma_start(
            out=masks[:].rearrange("p (j s) -> p j s", j=4),
            in_=masks_d.rearrange("j p s -> p j s"))
        nc.sync.dma_start(out=ones[:], in_=ones_d[:, :])

        # DRAM scratch: transposed activation master (residual stream)
        xmd = dpool.tile([NCH, 128, tok], F32, tag="xmd")

        # ---------------- steady-state pools ----------------
        pl = {}
        for nm, bufs, sp in (
                ("wk", 2, "SBUF"), ("wo", 2, "SBUF"), ("w1", 2, "SBUF"),
                ("w2", 2, "SBUF"), ("wv", 1, "SBUF"), ("qkT", 1, "SBUF"),
                ("vt", 1, "SBUF"), ("expT", 6, "SBUF"), ("oT", 1, "SBUF"),
                ("xubf", 1, "SBUF"), ("xbf", 1, "SBUF"), ("hbf", 1, "SBUF"),
                ("tmpn", 2, "SBUF"), ("xsq", 3, "SBUF"), ("tiny", 1, "SBUF"),
                ("xmp", 2, "SBUF"), ("ytp", 2, "SBUF"),
                ("proj", 2, "PSUM"), ("sc", 2, "PSUM"), ("ops", 1, "PSUM")):
            pl[nm] = stk.enter_context(tc.tile_pool(name=nm, bufs=bufs, space=sp))

        def layernorm(s0p, s1p, srow, brow, xu_list, xm_of):
            mean = pl["tiny"].tile([1, S], F32, tag="mean")
            var = pl["tiny"].tile([1, S], F32, tag="var")
            nm2 = pl["tiny"].tile([1, S], F32, tag="nm2")
            nc.vector.tensor_scalar(out=mean[:], in0=s0p[:], scalar1=1.0 / D,
                                    scalar2=None, op0=OP.mult)
            nc.vector.tensor_scalar(out=var[:], in0=s1p[:], scalar1=1.0 / D,
                                    scalar2=None, op0=OP.mult)
            # nm2 = -mean^2 ; var += nm2
            nc.vector.scalar_tensor_tensor(
                out=nm2[:], in0=mean[:], scalar=-1.0, in1=mean[:],
                op0=OP.mult, op1=OP.mult)
            nc.vector.tensor_add(var[:], var[:], nm2[:])
            std = pl["tiny"].tile([1, S], F32, tag="std")
            nc.scalar.activation(std[:], var[:], AF.Sqrt, bias=eps_t[:])
            a_f = pl["tiny"].tile([1, S], F32, tag="a_f")
            nc.vector.reciprocal(a_f[:], std[:])
            a_bf = pl["tiny"].tile([1, S], BF, tag="a_bf")
            nc.vector.tensor_copy(a_bf[:], a_f[:])
            b_bf = pl["tiny"].tile([1, S], BF, tag="b_bf")
            with nc.allow_low_precision(reason="bf16 LN shift"):
                nc.vector.scalar_tensor_tensor(
                    out=b_bf[:], in0=mean[:], scalar=-1.0, in1=a_f[:],
                    op0=OP.mult, op1=OP.mult)
            for c in range(NCH):
                Ap = lnps.tile([128, S], F32, tag="lnp")
                Bp = lnps.tile([128, S], F32, tag="lnp")
                nc.tensor.matmul(Ap[:], srow[0:1, c * 128:(c + 1) * 128],
                                 a_bf[:], start=True, stop=True,
                                 skip_group_check=True)
                nc.tensor.matmul(Bp[:], srow[0:1, c * 128:(c + 1) * 128],
                                 b_bf[:], start=True, stop=False,
                                 skip_group_check=True)
                nc.tensor.matmul(Bp[:], brow[0:1, c * 128:(c + 1) * 128],
                                 ones[0:1, :], start=False, stop=True,
                                 skip_group_check=True)
                t1 = pl["tmpn"].tile([128, S], F32, tag="tmpn")
                nc.vector.tensor_mul(t1[:], xu_list[c][:], Ap[:])
                nc.vector.tensor_add(xm_of(c), t1[:], Bp[:])

        def proj_ln(wpool, wsrc, nci, rhs_tiles, bcol, srow, brow, xm_of):
            s0p = lnps.tile([1, S], F32, tag="lnp")
            s1p = lnps.tile([1, S], F32, tag="lnp")
            xus = {}
            for oc in range(NCH):
                wt = wpool.tile([128, nci * 128], BF, tag="w")
                nc.sync.dma_start(
                    out=wt[:].rearrange("p (c m) -> p c m", c=nci),
                    in_=wsrc[oc].rearrange("c p m -> p c m"))
                p = pl["proj"].tile([128, S], F32, tag="proj")
                for kc in range(nci):
                    nc.tensor.matmul(
                        p[:], wt[:, kc * 128:(kc + 1) * 128], rhs_tiles[kc],
                        start=(kc == 0), stop=(kc == nci - 1))
                xu = pl["xubf"].tile([128, S], BF, tag=f"xu{oc}")
                with nc.allow_low_precision(reason="bf16 residual staging"):
                    nc.vector.scalar_tensor_tensor(
                        out=xu[:], in0=p[:], scalar=bcol[:, oc:oc + 1],
                        in1=xm_of(oc), op0=OP.add, op1=OP.add)
                xsq = pl["xsq"].tile([128, S], BF, tag="xsq")
                nc.vector.tensor_mul(xsq[:], xu[:], xu[:])
                nc.tensor.matmul(s0p[:], ones[:, 0:1], xu[:],
                                 start=(oc == 0), stop=(oc == NCH - 1),
                                 skip_group_check=True)
                nc.tensor.matmul(s1p[:], ones[:, 0:1], xsq[:],
                                 start=(oc == 0), stop=(oc == NCH - 1),
                                 skip_group_check=True)
                xus[oc] = xu
            layernorm(s0p, s1p, srow, brow, xus, xm_of)

        for li in range(L_run):
            # ---- per-layer params (host-prebuilt tiles) ----
            pcol = lpar.tile([128, 40], F32, tag="pcol")
            nc.sync.dma_start(out=pcol[:], in_=pcol_d[li])
            prow = lpar.tile([1, 4 * D], BF, tag="prow")
            nc.sync.dma_start(out=prow[:], in_=prow_d[li])
            bvb = lpar.tile([128, D], BF, tag="bvb")
            nc.sync.dma_start(out=bvb[:], in_=bvb_d[li])

            wv_t = pl["wv"].tile([128, NCH * S], BF, tag="wv")
            wv_t2 = pl["wv"].tile([128, NCH * S], BF, tag="wv2")
            for dc in range(NCH):
                nc.sync.dma_start(out=wv_t[:, dc * S:(dc + 1) * S],
                                  in_=wvbf[li, dc, :, 0:S])
                nc.sync.dma_start(out=wv_t2[:, dc * S:(dc + 1) * S],
                                  in_=wvbf[li, dc, :, S:D])


            for bi in range(BL_run):
                tb = bi * S
                # ---- stream in residual master + yT slices for this batch ----
                xmp = pl["xmp"].tile([128, NCH * S], F32, tag="xmp")
                xsrc = qT_d if li == 0 else xmd
                nc.sync.dma_start(
                    out=xmp[:].rearrange("p (c s) -> p c s", c=NCH),
                    in_=xsrc[:, :, tb:tb + S].rearrange("c p s -> p c s"))
                ytp = pl["ytp"].tile([128, NCH * S], BF, tag="ytp")
                nc.sync.dma_start(
                    out=ytp[:].rearrange("p (c s) -> p c s", c=NCH),
                    in_=qaT_d[:, :, tb:tb + S].rearrange("c p s -> p c s"))

                def xm_of(c):
                    return xmp[:, c * S:(c + 1) * S]

                # ---- qk projection ----
                xbf = []
                for c in range(NCH):
                    xb = pl["xbf"].tile([128, S], BF, tag=f"xbf{c}")
                    nc.scalar.activation(xb[:], xm_of(c), AF.Identity)
                    xbf.append(xb[:])
                qkT = pl["qkT"].tile([128, NCH * S], BF, tag="qkT")
                for oc in range(NCH):
                    wt = pl["wk"].tile([128, NCH * 128], BF, tag="w")
                    nc.sync.dma_start(
                        out=wt[:].rearrange("p (c m) -> p c m", c=NCH),
                        in_=wkbf[li, oc].rearrange("c p m -> p c m"))
                    p = pl["proj"].tile([128, S], F32, tag="proj")
                    for kc in range(NCH):
                        nc.tensor.matmul(
                            p[:], wt[:, kc * 128:(kc + 1) * 128], xbf[kc],
                            start=(kc == 0), stop=(kc == NCH - 1))
                    nc.scalar.activation(
                        qkT[:, oc * S:(oc + 1) * S], p[:], AF.Identity,
                        bias=pcol[:, oc:oc + 1], scale=S4)

                if dbg == "qkT" and li == 0 and bi == 0:
                    dq = lpar.tile([128, NCH * S], F32, tag="dbgt")
                    nc.vector.tensor_copy(dq[:], qkT[:])
                    nc.sync.dma_start(out=dbg_d[:, :], in_=dq[:])
                # ---- v projection (natural layout, per-head + ones col) ----
                vt = pl["vt"].tile([128, 4 * H * 65], BF, tag="vt")
                nc.vector.memset(
                    vt[:].rearrange("p (t h e) -> p t h e", t=4, h=H)
                    [:, :, :, 64:65], 1.0)
                for tc4 in range(4):
                    for hf, wvx in ((0, wv_t), (1, wv_t2)):
                        p = pl["proj"].tile([128, S], F32, tag="proj")
                        for dc in range(NCH):
                            nc.tensor.matmul(
                                p[:],
                                ytp[:, dc * S + tc4 * 128:dc * S + tc4 * 128 + 128],
                                wvx[:, dc * S:(dc + 1) * S],
                                start=(dc == 0), stop=(dc == NCH - 1))
                        dst = (vt[:]
                               .rearrange("p (t h e) -> p t h e", t=4, h=H)
                               [:, tc4, hf * 8:(hf + 1) * 8, 0:64])
                        with nc.allow_low_precision(reason="bf16 v staging"):
                            nc.vector.tensor_add(
                                dst,
                                p[:].rearrange("p (h e) -> p h e", h=8),
                                bvb[:, hf * S:(hf + 1) * S]
                                .rearrange("p (h e) -> p h e", h=8))

                if dbg == "vt" and li == 0 and bi == 0:
                    dq = lpar.tile([128, NCH * S], F32, tag="dbgt")
                    nc.vector.tensor_copy(dq[:], vt[:, 0:NCH * S])
                    nc.sync.dma_start(out=dbg_d[:, :], in_=dq[:])
                # ---- attention (scores pipelined one head ahead of o) ----
                oT = pl["oT"].tile([128, NCH * S], BF, tag="oT")
                exps = {}

                def emit_scores(h):
                    qs = qkT[(h % 2) * 64:(h % 2) * 64 + 64,
                             (h // 2) * S:(h // 2) * S + S]
                    tiles = []
                    for jc in range(4):
                        sp = pl["sc"].tile([128, S], F32, tag="sc")
                        nc.tensor.matmul(
                            sp[:], qs[:, jc * 128:(jc + 1) * 128], qs,
                            start=True, stop=True)
                        nc.vector.tensor_add(
                            sp[:], sp[:], masks[:, jc * S:(jc + 1) * S])
                        et = pl["expT"].tile([128, S], BF, tag="expT")
                        nc.scalar.activation(et[:], sp[:], AF.Exp)
                        tiles.append(et)
                    exps[h] = tiles

                def emit_o(h):
                    tiles = exps.pop(h)
                    op_ = pl["ops"].tile([65, S], F32, tag="ops")
                    for jc in range(4):
                        nc.tensor.matmul(
                            op_[:],
                            vt[:].rearrange("p (t e) -> p t e", t=4)
                            [:, jc, h * 65:(h + 1) * 65],
                            tiles[jc][:], start=(jc == 0), stop=(jc == 3))
                    rzf = pl["tiny"].tile([1, S], F32, tag="rzf")
                    rz = pl["tiny"].tile([1, S], BF, tag="rz")
                    nc.vector.tensor_scalar(
                        out=rzf[:], in0=op_[64:65, :], scalar1=1e-30,
                        scalar2=None, op0=OP.add)
                    with nc.allow_low_precision(reason="bf16 softmax recip"):
                        nc.vector.reciprocal(rz[:], rzf[:])
                    rzb = lnps.tile([64, S], F32, tag="lnp")
                    nc.tensor.matmul(rzb[:], ones[0:1, 0:64], rz[:],
                                     start=True, stop=True,
                                     skip_group_check=True)
                    rzbs = pl["tmpn"].tile([64, S], BF, tag="rzbs")
                    nc.scalar.activation(rzbs[:], rzb[:], AF.Identity)
                    nc.vector.tensor_mul(
                        oT[(h % 2) * 64:(h % 2) * 64 + 64,
                           (h // 2) * S:(h // 2) * S + S],
                        op_[0:64, :], rzbs[:])

                emit_scores(0)
                for h in range(H):
                    if h + 1 < H:
                        emit_scores(h + 1)
                    emit_o(h)

                if dbg == "oT" and li == 0 and bi == 0:
                    dq = lpar.tile([128, NCH * S], F32, tag="dbgt")
                    nc.vector.tensor_copy(dq[:], oT[:])
                    nc.sync.dma_start(out=dbg_d[:, :], in_=dq[:])
                # ---- out projection + LN1 ----
                oT_tiles = [oT[:, c * S:(c + 1) * S] for c in range(NCH)]
                proj_ln(pl["wo"], wobf[li], NCH, oT_tiles,
                        pcol[:, 8:16], prow[0:1, 0:D], prow[0:1, D:2 * D],
                        xm_of)

                if dbg == "ln1" and li == 0 and bi == 0:
                    dq = lpar.tile([128, NCH * S], F32, tag="dbgt")
                    nc.vector.tensor_copy(dq[:], xmp[:])
                    nc.sync.dma_start(out=dbg_d[:, :], in_=dq[:])
                # ---- FFN ----
                xbf2 = []
                for c in range(NCH):
                    xb = pl["xbf"].tile([128, S], BF, tag=f"xbf{c}")
                    nc.scalar.activation(xb[:], xm_of(c), AF.Identity)
                    xbf2.append(xb[:])
                hb = pl["hbf"].tile([128, NFF * S], BF, tag="hb")
                for fc in range(NFF):
                    wt = pl["w1"].tile([128, NCH * 128], BF, tag="w")
                    nc.sync.dma_start(
                        out=wt[:].rearrange("p (c m) -> p c m", c=NCH),
                        in_=w1bf[li, fc].rearrange("c p m -> p c m"))
                    p = pl["proj"].tile([128, S], F32, tag="proj")
                    for kc in range(NCH):
                        nc.tensor.matmul(
                            p[:], wt[:, kc * 128:(kc + 1) * 128], xbf2[kc],
                            start=(kc == 0), stop=(kc == NCH - 1))
                    nc.scalar.activation(
                        hb[:, fc * S:(fc + 1) * S], p[:], AF.Relu,
                        bias=pcol[:, 16 + fc:17 + fc])
                hb_tiles = [hb[:, f * S:(f + 1) * S] for f in range(NFF)]
                proj_ln(pl["w2"], w2bf[li], NFF, hb_tiles,
                        pcol[:, 32:40], prow[0:1, 2 * D:3 * D],
                        prow[0:1, 3 * D:4 * D], xm_of)

                # ---- write back residual master ----
                wdst = out_d if li == L_run - 1 else xmd
                nc.sync.dma_start(
                    out=wdst[:, :, tb:tb + S].rearrange("c p s -> p c s"),
                    in_=xmp[:].rearrange("p (c s) -> p c s", c=NCH))

    return nc


_host_consts = None


def host_consts():
    global _host_consts
    if _host_consts is None:
        m = np.full((4, 128, S), NEG, np.float32)
        for jc in range(4):
            j = jc * 128 + np.arange(128)[:, None]
            i = np.arange(S)[None, :]
            m[jc][i > j] = 0.0
        _host_consts = {
            "masks": m.astype(ml_dtypes.bfloat16),
            "ones": np.ones((128, S), ml_dtypes.bfloat16),
        }
    return _host_consts


def prep_weights(inputs):
    """Host-side: cast weights to bf16 and pre-tile into lhsT layouts."""
    BFh = ml_dtypes.bfloat16
    Wk, Wo = inputs["Wk"], inputs["Wo"]
    W1, W2, Wv = inputs["W1"], inputs["W2"], inputs["Wv"]
    wk_t = np.ascontiguousarray(
        Wk.reshape(L, NCH, 128, NCH, 128).transpose(0, 3, 1, 2, 4)).astype(BFh)
    wo_t = np.ascontiguousarray(
        Wo.reshape(L, NCH, 128, NCH, 128).transpose(0, 3, 1, 2, 4)).astype(BFh)
    w1_t = np.ascontiguousarray(
        W1.reshape(L, NCH, 128, NFF, 128).transpose(0, 3, 1, 2, 4)).astype(BFh)
    w2_t = np.ascontiguousarray(
        W2.reshape(L, NFF, 128, NCH, 128).transpose(0, 3, 1, 2, 4)).astype(BFh)
    wv_t = np.ascontiguousarray(Wv.reshape(L, NCH, 128, D)).astype(BFh)
    BFh = ml_dtypes.bfloat16
    pcol = np.zeros((L, 128, 40), np.float32)
    pcol[:, :, 0:8] = (inputs["bk"] * S4).reshape(L, NCH, 128).transpose(0, 2, 1)
    pcol[:, :, 8:16] = inputs["bo"].reshape(L, NCH, 128).transpose(0, 2, 1)
    pcol[:, :, 16:32] = inputs["b1"].reshape(L, NFF, 128).transpose(0, 2, 1)
    pcol[:, :, 32:40] = inputs["b2"].reshape(L, NCH, 128).transpose(0, 2, 1)
    prow = np.concatenate([inputs["ln1_s"], inputs["ln1_b"],
                           inputs["ln2_s"], inputs["ln2_b"]],
                          axis=1).reshape(L, 1, 4 * D).astype(BFh)
    bvb = np.broadcast_to(
        inputs["bv"].astype(BFh)[:, None, :], (L, 128, D)).copy()
    return {"wk_t": wk_t, "wo_t": wo_t, "w1_t": w1_t, "w2_t": w2_t,
            "wv_t": wv_t, "pcol_h": np.ascontiguousarray(pcol),
            "prow_h": np.ascontiguousarray(prow),
            "bvb_h": np.ascontiguousarray(bvb)}


def embedT(x, tok):
    # [tok, D] -> [NCH, 128, tok]
    return np.ascontiguousarray(x.reshape(tok, NCH, 128).transpose(1, 2, 0))


def make_in_maps(inputs, ncores=NCORES):
    hc = host_consts()
    shared = prep_weights(inputs)
    shared.update(hc)
    qf = inputs["q_embed"].reshape(ncores, -1, D)
    qaf = inputs["qa_embed"].reshape(ncores, -1, D)
    tok = qf.shape[1]
    in_maps = []
    for c in range(ncores):
        im = {"qT": embedT(qf[c], tok),
              "qaT": embedT(qaf[c], tok).astype(ml_dtypes.bfloat16)}
        im.update(shared)
        in_maps.append(im)
    return in_maps


def finalize_waits(nc):
    """Split multi-sem waits to satisfy TRN2 1-wait-per-instruction limit."""
    from concourse.bass_utils import bass_rust
    bass_rust.move_matmul_waits_to_ldweights(nc.m)
    bass_rust.generate_event_semaphores(nc)


def kernel(**inputs):
    inputs = {k: np.ascontiguousarray(np.asarray(v)) for k, v in inputs.items()}
    nc = bass.Bass(trn_type="TRN2")
    build(nc)
    finalize_waits(nc)
    in_maps = make_in_maps(inputs)
    res = run_bass_kernel_spmd(nc, in_maps, list(range(NCORES)))
    # out: [NCH, 128, TOK] transposed layout -> [TOK, D] -> [B, S, D]
    outs = []
    for c in range(NCORES):
        o = res.results[c]["out"]
        outs.append(o.transpose(2, 0, 1).reshape(TOK, D))
    return np.stack(outs).reshape(B, S, D).astype(np.float32)

